# revision 1
# baseline (speedup 1.0000x reference)
"""Trainium2 Bass kernel for nn_Attention_54150947668207 (sparse channel attention).

Algorithm restructure (verified exact vs reference in fp32, rel 3.4e-7):
  - dwconv3x3 per channel on q,k,v (depthwise, SAME pad)
  - per (batch,head): attn = normalize(q) @ normalize(k)^T over pixels; the 4
    top-k masked softmaxes combine into ONE matrix A_comb = sum_i w_i*softmax_i
    (top-k via rank-count, col-scaled before ranking; exp needs no max-sub)
  - M_combT = blockdiag(A_comb)^T @ Wproj^T folds all four attn@v matmuls AND
    the 1x1 projection into ONE [384,384]@[384,px] matmul per pixel shard.

Sharding (8 cores, SPMD):
  - attention phase: core i handles batch i//4, heads {2*(i%4), 2*(i%4)+1}
  - projection phase: core i handles batch i//4, image rows 32*(i%4)..+32
  - connected by one AllGather of M_combT ([96,384] f16) in 4-core groups.

Implementation notes (measured on HW):
  - fp16 on device (inputs host-cast); matmuls accumulate fp32 in PSUM.
  - dwconv: 6 taps as PE diag-matmuls (fp32 PSUM acc, includes all odd-column
    taps), 3 taps on DVE as tensor_scalar(4x) + tensor_tensor(2x) — the fused
    scalar_tensor_tensor runs 1x-only on this firmware, and ScalarE is 1x for
    everything, so work is kept off ACT except PSUM drains/squares/exp.
  - PE HAM warmth: dense MM stream + warm-keeper matmuls bridging the
    collective window; transposes issued from ACT to keep the SP DMA queue
    pure-loads; independent pipelines use private tile-pool tags.
  - walrus here accepts ONE sync wait per instruction: _split_multi_waits
    rewrites Tile's multi-wait instructions into NoOp-carried single waits.
  - the LAST q/k chunk runs all 9 taps on PE (fp32 PSUM): removes the serial
    DVE tap chain from the critical path into the tail and cuts rounding on
    the rank-sensitive attention inputs.
HW exec ~304us/8 cores (neuron-profile), rel err 9.1e-3 vs fp32 reference.
"""
import sys

for _p in ("/opt/trn_rl_repo",):
    if _p not in sys.path:
        sys.path.insert(0, _p)

import numpy as np
from contextlib import ExitStack

import concourse.bass as bass
import concourse.tile as tile
from concourse import mybir
from concourse.bass_utils import run_bass_kernel_spmd

F32 = mybir.dt.float32
F16 = mybir.dt.float16
AOT = mybir.AluOpType
ACTF = mybir.ActivationFunctionType

C = 384
HEADS = 8
CH = 48          # channels per head
H = W = 128
HW = H * W
B = 2
C2 = 96          # channels per core in attention phase (2 heads)
KS = (CH // 2, CH * 2 // 3, CH * 3 // 4, CH * 4 // 5)  # 24, 32, 36, 38

# tap order t = 3*ky + kx, offsets (dy,dx) = (ky-1, kx-1)
N_TAPS = 9
CFG = {
    "pe_taps_qk": (0, 1, 3, 4, 5, 7),   # taps done on PE (fp32 psum acc)
    "pe_taps_v": (0, 1, 3, 4, 5, 7),
    "big_xpose": True,               # one dma_start_transpose per chunk
    "dve_style": "ts_tt",            # "stt" | "ts_tt" | "act_tt"
    "split_waits": True,
    "max_waits": 1,
}

NCHUNK = 4            # q/k processed in 4 chunks of 32 rows
ROWS_PER_CHUNK = 32
CHUNK_PX = ROWS_PER_CHUNK * W   # 4096
SUB = 512             # psum sub-chunk width for PE dwconv
VROWS = 32            # v shard rows per core
VPX = VROWS * W       # 4096


def _split_multi_waits(nc, max_waits=1):
    """walrus in this container accepts limited sync waits per instruction;
    split extras into preceding single-wait NoOps on the same engine."""
    n = 0
    for f in nc.m.functions:
        for blk in f.blocks:
            new_insts = []
            for inst in blk.instructions:
                si = getattr(inst, "sync_info", None)
                if si is not None and si.on_wait and len(si.on_wait) > max_waits:
                    waits = list(si.on_wait)
                    for wcond in waits[:-max_waits]:
                        nop = mybir.InstNoOp(
                            name=f"I-waitsplit-{nc.next_id()}",
                            ins=[], outs=[],
                            engine=inst.engine,
                            sync_info=mybir.SyncInfo(on_wait=[wcond], on_update=[]),
                        )
                        new_insts.append(nop)
                        n += 1
                    si.on_wait = waits[-max_waits:]
                new_insts.append(inst)
            blk.instructions = new_insts
    return n


def _emit_dwconv(nc, pool, psum_dw, xpad, w9, diags, dwp, out_tag,
                 npart, pe_taps, nrows, out_final=None):
    """Depthwise 3x3 over nrows output rows.

    PE taps accumulate in fp32 PSUM (diag matmuls); remaining taps run as
    ACT per-partition-scaled copies + DVE tensor_tensor adds (both 2x/4x
    capable, unlike SCALAR_TENSOR_TENSOR which runs 1x on this fw).
    Returns the final output AP ([npart, nrows*W] f16).
    """
    dve_taps = [t for t in range(N_TAPS) if t not in pe_taps]
    npx = nrows * W
    nsub = npx // 1024
    rows_per_sub = 1024 // W  # 8

    def shifted(t, r_lo, nr):
        ky, kx = divmod(t, 3)
        return xpad[:, r_lo + ky:r_lo + ky + nr, kx:kx + W]

    _ctr = [0]

    def alloc(tag):
        _ctr[0] += 1
        t = dwp.tile([npart, npx], F16, tag=tag, name=f"{out_tag}_{tag}{_ctr[0]}")
        return t[:]

    n_dve = len(dve_taps)
    assert pe_taps
    cur = alloc(out_tag + "A")
    oc3 = cur.rearrange("p (r w) -> p r w", w=W)
    for s in range(nsub):
        r_lo = s * rows_per_sub
        ps = psum_dw.tile([npart, 1024], F32, tag="psdw")
        for half in range(2):
            for i, t in enumerate(pe_taps):
                nc.tensor.matmul(
                    ps[:, half * 512:half * 512 + 512], diags[t],
                    shifted(t, r_lo + half * 4, 4),
                    start=(i == 0), stop=(i == len(pe_taps) - 1))
        nc.scalar.copy(oc3[:, r_lo:r_lo + rows_per_sub, :], ps[:])
    if n_dve == 0:
        return cur
    style = CFG["dve_style"]
    flip = 1
    for j, t in enumerate(dve_taps):
        last = (j == n_dve - 1)
        nxt = out_final if (last and out_final is not None) else alloc(out_tag + "BA"[flip])
        flip ^= 1
        no3 = nxt.rearrange("p (r w) -> p r w", w=W)
        if style == "stt":
            nc.vector.scalar_tensor_tensor(
                no3, shifted(t, 0, nrows), w9[:, t:t + 1], oc3, AOT.mult, AOT.add)
        else:
            tmp = alloc("dwtmp")
            tm3 = tmp.rearrange("p (r w) -> p r w", w=W)
            if style == "act_tt":
                nc.scalar.mul(tm3, shifted(t, 0, nrows), w9[:, t:t + 1])
            else:
                nc.vector.tensor_scalar(
                    tm3, shifted(t, 0, nrows), w9[:, t:t + 1], None, AOT.mult)
            nc.vector.tensor_tensor(no3, tm3, oc3, AOT.add)
        cur, oc3 = nxt, no3
    return cur


def build_kernel():
    nc = bass.Bass("TRN2", target_bir_lowering=False, debug=False, num_devices=8)

    # ---- DRAM I/O ----
    qs = nc.declare_dram_parameter("qs", [C2, 130, 130], F16, isOutput=False)
    ks = nc.declare_dram_parameter("ks", [C2, 130, 130], F16, isOutput=False)
    vs = nc.declare_dram_parameter("vs", [C, 34, 130], F16, isOutput=False)
    wq9 = nc.declare_dram_parameter("wq9", [C2, 9], F32, isOutput=False)
    wk9 = nc.declare_dram_parameter("wk9", [C2, 9], F32, isOutput=False)
    wv9 = nc.declare_dram_parameter("wv9", [C, 9], F32, isOutput=False)
    NPEV = len(CFG["pe_taps_v"])
    dgqk = nc.declare_dram_parameter("dgqk", [2, 9, C2, C2], F16, isOutput=False)
    dgv = nc.declare_dram_parameter("dgv", [NPEV, 3, 128, 128], F16, isOutput=False)
    wpT0 = nc.declare_dram_parameter("wpT0", [CH, C], F16, isOutput=False)
    wpT1 = nc.declare_dram_parameter("wpT1", [CH, C], F16, isOutput=False)
    tempv = nc.declare_dram_parameter("tempv", [C2, 1], F32, isOutput=False)
    attwv = nc.declare_dram_parameter("attwv", [C2, 4], F32, isOutput=False)
    out_ext = nc.declare_dram_parameter("out", [3, 128, VPX], F16, isOutput=True)

    with tile.TileContext(nc) as tc, ExitStack() as ctx:
        pool = ctx.enter_context(tc.tile_pool(name="sbuf", bufs=1))
        pads = ctx.enter_context(tc.tile_pool(name="pads", bufs=3))
        dwp = ctx.enter_context(tc.tile_pool(name="dwp", bufs=2))
        psum_dw = ctx.enter_context(tc.tile_pool(name="psdw", bufs=2, space="PSUM"))
        psum_a = ctx.enter_context(tc.tile_pool(name="psa", bufs=1, space="PSUM"))
        psum_o = ctx.enter_context(tc.tile_pool(name="pso", bufs=2, space="PSUM"))
        obuf = ctx.enter_context(tc.tile_pool(name="obuf", bufs=3))
        dram = ctx.enter_context(tc.tile_pool(name="dram", bufs=1, space="DRAM"))

        # ---- constants ----
        w9q = pool.tile([C2, 9], F32); nc.sync.dma_start(w9q[:], wq9.ap())
        w9k = pool.tile([C2, 9], F32); nc.sync.dma_start(w9k[:], wk9.ap())
        w9v = pool.tile([128, 3, 9], F32)
        for ct in range(3):
            nc.sync.dma_start(w9v[:, ct, :], wv9.ap()[128 * ct:128 * (ct + 1), :])
        dgqk_t = pool.tile([C2, 2, 9, C2], F16, tag="dgqk")
        nc.sync.dma_start(dgqk_t[:], dgqk.ap().rearrange("a t c e -> c a t e"))
        dgv_t = pool.tile([128, NPEV, 3, 128], F16, tag="dgvt")
        nc.sync.dma_start(dgv_t[:], dgv.ap().rearrange("t g c e -> c t g e"))
        diag_q = {t: dgqk_t[:, 0, t, :] for t in range(9)}
        diag_k = {t: dgqk_t[:, 1, t, :] for t in range(9)}
        diag_v = {(t, ct): dgv_t[:, i, ct, :]
                  for i, t in enumerate(CFG["pe_taps_v"]) for ct in range(3)}
        wp0 = pool.tile([CH, C], F16); nc.sync.dma_start(wp0[:], wpT0.ap())
        wp1 = pool.tile([CH, C], F16); nc.sync.dma_start(wp1[:], wpT1.ap())
        tmpv = pool.tile([C2, 1], F32); nc.sync.dma_start(tmpv[:], tempv.ap())
        attw = pool.tile([C2, 4], F32); nc.sync.dma_start(attw[:], attwv.ap())

        # ---- q/k dwconv + transpose, interleaved by chunk; attn MMs per chunk ----
        sumsq = {}
        vdw = pool.tile([128, 3, VPX], F16, tag="vdw")
        qT = pool.tile([128, 128, C2], F16, tag="qT")
        kT = pool.tile([128, 128, C2], F16, tag="kT")
        ps_attn = psum_a.tile([C2, C2], F32, tag="psattn")
        for ci in range(NCHUNK):
            r0 = ci * ROWS_PER_CHUNK
            xq = pads.tile([C2, 34, 130], F16, tag="pad", name=f"xq{ci}")
            nc.sync.dma_start(xq[:], qs.ap()[:, r0:r0 + 34, :])
            xk = pads.tile([C2, 34, 130], F16, tag="pad", name=f"xk{ci}")
            nc.sync.dma_start(xk[:], ks.ap()[:, r0:r0 + 34, :])
            dws = {}
            for name, xpad, w9, diags in (("q", xq, diag_q, None), ("k", xk, diag_k, None)):
                w9_ = w9q if name == "q" else w9k
                dg_ = diag_q if name == "q" else diag_k
                taps = CFG["pe_taps_qk"] if ci < NCHUNK - 1 else tuple(range(9))
                dw = _emit_dwconv(nc, pool, psum_dw, xpad, w9_, dg_,
                                  dwp, "dw", C2, taps, ROWS_PER_CHUNK)
                dws[name] = dw
                sq = dwp.tile([C2, CHUNK_PX], F16, tag="dwtmp", name=f"sq_{name}{ci}")
                ss = pool.tile([C2, 1], F32, tag=f"ss_{name}{ci}")
                nc.scalar.activation(sq[:], dw, ACTF.Square, accum_out=ss[:])
                sumsq[(name, ci)] = ss
            xp_eng = nc.scalar if ci < NCHUNK - 1 else nc.sync
            xp_eng.dma_start_transpose(qT[:, 32 * ci:32 * ci + 32, :], dws["q"])
            xp_eng.dma_start_transpose(kT[:, 32 * ci:32 * ci + 32, :], dws["k"])
            # attention matmuls for this chunk's 32 pixel groups
            for j in range(32 * ci, 32 * ci + 32):
                nc.tensor.matmul(ps_attn[:], qT[:, j, :], kT[:, j, :],
                                 start=(j == 0), stop=(j == 127))
            if ci == 0:
                # v dwconv scheduled after the first q/k chunk is in flight
                for ct in range(3):
                    vp = pads.tile([128, 34, 130], F16, tag="pad")
                    nc.sync.dma_start(vp[:], vs.ap()[128 * ct:128 * (ct + 1), :, :])
                    diags_v = {t: diag_v[(t, ct)] for t in CFG["pe_taps_v"]}
                    _emit_dwconv(nc, pool, psum_dw, vp, w9v[:, ct, :],
                                 diags_v, dwp, "vw", 128, CFG["pe_taps_v"], VROWS,
                                 out_final=vdw[:, ct, :])

        # total sumsq -> [C2,1]
        nq2 = pool.tile([C2, 1], F32, tag="nq2")
        nk2 = pool.tile([C2, 1], F32, tag="nk2")
        for name, tgt in (("q", nq2), ("k", nk2)):
            nc.vector.tensor_tensor(
                tgt[:], sumsq[(name, 0)][:], sumsq[(name, 1)][:], AOT.add)
            nc.vector.tensor_tensor(
                tgt[:], tgt[:], sumsq[(name, 2)][:], AOT.add)
            nc.vector.tensor_tensor(
                tgt[:], tgt[:], sumsq[(name, 3)][:], AOT.add)

        # warm-keeper: junk matmuls with a fake dep on `gate` keep HAM at 2.4GHz
        warm_ps = psum_a.tile([C2, C2], F32, tag="psattn", name="warmps")
        warm_ctr = [0]

        def warm(gate, n=2):
            m = min(gate.shape[-1], C2)
            for _ in range(n):
                warm_ctr[0] += 1
                nc.tensor.matmul(warm_ps[0:m, 0:m], gate[:, 0:m], gate[:, 0:m],
                                 start=True, stop=True)

        # ---- post-attention (small) ----
        # A1[r, d] = attn[r, head(r)*48 + d]; head0 block straight from psum,
        # head1 block via an SBUF bounce (DMA cannot read PSUM, ACT cannot
        # start at partition 48)
        A1 = pool.tile([C2, CH], F32, tag="A1")
        A0 = pool.tile([C2, C2], F32, tag="A0")
        nc.scalar.copy(A1[0:CH, :], ps_attn[0:CH, 0:CH])
        nc.scalar.copy(A0[:], ps_attn[:])
        nc.sync.dma_start(A1[CH:C2, :], A0[CH:C2, CH:C2])
        # rq = 1/sqrt(nq2), rk = 1/sqrt(nk2)
        rq = pool.tile([C2, 1], F32, tag="rq")
        rk = pool.tile([C2, 1], F32, tag="rk")
        for src2, dst in ((nq2, rq), (nk2, rk)):
            nc.scalar.sqrt(dst[:], src2[:])
            nc.vector.reciprocal(dst[:], dst[:])
        # column scaling by rk: build B [C2, CH]: rows 0:48 = rk[0:48]^T, rows 48:96 = rk[48:96]^T
        rk_dram = dram.tile([C2, 1], F32)
        nc.sync.dma_start(rk_dram[:], rk[:])
        Bc = pool.tile([C2, CH], F32, tag="Bc")
        rkd = rk_dram[:].rearrange("p one -> (p one)")
        nc.sync.dma_start(
            Bc[0:CH, :],
            rkd[0:CH].rearrange("(x e) -> x e", x=1).broadcast_to([CH, CH]))
        nc.sync.dma_start(
            Bc[CH:C2, :],
            rkd[CH:C2].rearrange("(x e) -> x e", x=1).broadcast_to([CH, CH]))
        nc.vector.tensor_tensor(A1[:], A1[:], Bc[:], AOT.mult)
        warm(A1[:])
        # rank count: G[r, d, e] = A1[r, e] > A1[r, d]  (free dims d,e)
        G = pool.tile([C2, CH, CH], F16, tag="G")
        in_e = A1[:].rearrange("p (x e) -> p x e", x=1).broadcast_to([C2, CH, CH])
        in_d = A1[:].rearrange("p (d x) -> p d x", x=1).broadcast_to([C2, CH, CH])
        nc.vector.tensor_tensor(G[:], in_e, in_d, AOT.is_gt)
        cnt = pool.tile([C2, CH], F32, tag="cnt")
        nc.vector.tensor_reduce(cnt[:], G[:], axis=mybir.AxisListType.X, op=AOT.add)
        warm(cnt[:])
        # row scale = temp * rq; exp
        rsc = pool.tile([C2, 1], F32, tag="rsc")
        nc.vector.tensor_tensor(rsc[:], rq[:], tmpv[:], AOT.mult)
        E = pool.tile([C2, CH], F32, tag="E")
        nc.scalar.activation(E[:], A1[:], ACTF.Exp, scale=rsc[:])
        warm(E[:])
        # 4 masked softmaxes combined
        Acc = pool.tile([C2, CH], F32, tag="Acc")
        numer = pool.tile([C2, CH], F32, tag="numer")
        for i, kk in enumerate(KS):
            mi = pool.tile([C2, CH], F16, tag="mi")
            nc.vector.tensor_scalar(mi[:], cnt[:], float(kk), None, AOT.is_lt)
            nc.vector.tensor_tensor(numer[:], E[:], mi[:], AOT.mult)
            den = pool.tile([C2, 1], F32, tag="den")
            nc.vector.tensor_reduce(den[:], numer[:], axis=mybir.AxisListType.X,
                                    op=AOT.add)
            rw = pool.tile([C2, 1], F32, tag="rw")
            nc.vector.reciprocal(rw[:], den[:])
            nc.vector.tensor_tensor(rw[:], rw[:], attw[:, i:i + 1], AOT.mult)
            if i == 0:
                nc.vector.tensor_scalar(Acc[:], numer[:], rw[:], None, AOT.mult)
            else:
                nc.vector.scalar_tensor_tensor(
                    Acc[:], numer[:], rw[:], Acc[:], AOT.mult, AOT.add)
            warm(numer[:])
        # per-head A_comb f16 tiles (base_partition 0 for lhsT)
        Ah = []
        for h in range(2):
            a = pool.tile([CH, CH], F16, tag=f"Ah{h}")
            nc.gpsimd.dma_start(a[:], Acc[CH * h:CH * (h + 1), :])
            Ah.append(a)
            warm(a[:])

        # ---- M_combT partial: per head [48, 384] ----
        b_in = dram.tile([C2, C], F16)
        b_out = dram.tile([4, C2, C], F16)
        for h, wp in enumerate((wp0, wp1)):
            ps = psum_a.tile([CH, C], F32, tag="psmc")
            nc.tensor.matmul(ps[:], Ah[h][:], wp[:], start=True, stop=True)
            mt_h = pool.tile([CH, C], F16, tag=f"mth{h}")
            nc.scalar.copy(mt_h[:], ps[:])
            nc.sync.dma_start(b_in[CH * h:CH * (h + 1), :], mt_h[:])
            warm(mt_h[:], n=3)

        # ---- AllGather M_combT within 4-core groups ----
        nc.gpsimd.collective_compute(
            "AllGather", AOT.bypass,
            replica_groups=[[0, 1, 2, 3], [4, 5, 6, 7]],
            ins=[b_in.opt()], outs=[b_out.opt()])
        MT = pool.tile([128, 3, C], F16, tag="MT")
        bo = b_out[:].rearrange("g p c -> (g p) c")
        for kc in range(3):
            nc.sync.dma_start(MT[:, kc, :], bo[128 * kc:128 * (kc + 1), :])

        # ---- final matmul: out[o, px] = sum_c MT[c, o] * vdw[c, px] ----
        for m in range(3):
            for n in range(VPX // SUB):
                ps = psum_o.tile([128, SUB], F32, tag="psout")
                for kc in range(3):
                    nc.tensor.matmul(
                        ps[:], MT[:, kc, 128 * m:128 * (m + 1)],
                        vdw[:, kc, SUB * n:SUB * (n + 1)],
                        start=(kc == 0), stop=(kc == 2))
                ob = obuf.tile([128, SUB], F16, tag="ob")
                nc.scalar.copy(ob[:], ps[:])
                nc.sync.dma_start(out_ext.ap()[m, :, SUB * n:SUB * (n + 1)], ob[:])

    if CFG["split_waits"]:
        _split_multi_waits(nc, CFG["max_waits"])
    return nc


# ---------------- host-side sharding ----------------

def _prep_inputs(q_fea, k_fea, v_fea, wq, wk, wv, wproj, temperature, attn_w):
    q_fea = np.asarray(q_fea, np.float32)
    k_fea = np.asarray(k_fea, np.float32)
    v_fea = np.asarray(v_fea, np.float32)
    wq = np.asarray(wq, np.float32)[:, 0]      # [C,3,3]
    wk = np.asarray(wk, np.float32)[:, 0]
    wv = np.asarray(wv, np.float32)[:, 0]
    wproj = np.asarray(wproj, np.float32)[:, :, 0, 0]  # [C,C]
    temp = np.asarray(temperature, np.float32).reshape(HEADS)
    attn_w = np.asarray(attn_w, np.float32).reshape(4)

    wq9 = wq.reshape(C, 9)
    wk9 = wk.reshape(C, 9)
    wv9 = wv.reshape(C, 9)

    pe_qk = CFG["pe_taps_qk"]
    pe_v = CFG["pe_taps_v"]
    dgv = np.zeros((len(pe_v), 3, 128, 128), np.float16)
    for i, t in enumerate(pe_v):
        for ct in range(3):
            np.fill_diagonal(dgv[i, ct], wv9[128 * ct:128 * (ct + 1), t].astype(np.float16))

    in_maps = []
    for core in range(8):
        b = core // 4
        g = core % 4
        ch0 = C2 * g
        r0 = VROWS * g

        def padqk(x):
            p = np.zeros((C2, 130, 130), np.float16)
            p[:, 1:129, 1:129] = x[b, ch0:ch0 + C2]
            return p

        vp = np.zeros((C, 34, 130), np.float16)
        glo = max(0, r0 - 1)
        ghi = min(H, r0 + VROWS + 1)
        vp[:, glo - (r0 - 1):ghi - (r0 - 1), 1:129] = v_fea[b, :, glo:ghi, :]

        dgqk = np.zeros((2, 9, C2, C2), np.float16)
        for t in range(9):
            np.fill_diagonal(dgqk[0, t], wq9[ch0:ch0 + C2, t].astype(np.float16))
            np.fill_diagonal(dgqk[1, t], wk9[ch0:ch0 + C2, t].astype(np.float16))

        in_maps.append({
            "qs": padqk(q_fea),
            "ks": padqk(k_fea),
            "vs": vp,
            "wq9": np.ascontiguousarray(wq9[ch0:ch0 + C2]),
            "wk9": np.ascontiguousarray(wk9[ch0:ch0 + C2]),
            "wv9": wv9,
            "dgqk": dgqk,
            "dgv": dgv,
            "wpT0": np.ascontiguousarray(wproj[:, ch0:ch0 + CH].T.astype(np.float16)),
            "wpT1": np.ascontiguousarray(wproj[:, ch0 + CH:ch0 + C2].T.astype(np.float16)),
            "tempv": np.repeat(temp[2 * g + HEADS * 0:2 * g + 2], CH)[:, None].copy(),
            "attwv": np.tile(attn_w, (C2, 1)),
        })
    return in_maps


def _assemble(results):
    out = np.zeros((B, C, H, W), np.float32)
    for core in range(8):
        b = core // 4
        r0 = VROWS * (core % 4)
        o = results[core]["out"].astype(np.float32)  # [3, 128, VPX]
        out[b, :, r0:r0 + VROWS, :] = o.reshape(C, VROWS, W)
    return out


_CACHE = {}


def kernel(**inputs) -> np.ndarray:
    if "nc" not in _CACHE:
        _CACHE["nc"] = build_kernel()
    nc = _CACHE["nc"]
    in_maps = _prep_inputs(**inputs)
    res = run_bass_kernel_spmd(nc, in_maps, core_ids=list(range(8)))
    return _assemble(res.results)


if __name__ == "__main__":
    sys.path.insert(0, "/root/problem")
    from reference import setup_inputs, reference

    inputs = setup_inputs()
    ref = np.asarray(reference(**inputs))
    got = kernel(**{k: np.asarray(v) for k, v in inputs.items()})
    rel = np.linalg.norm(got - ref) / np.linalg.norm(ref)
    print(f"Relative error: {rel:.3e}")



# revision 20
# speedup vs baseline: 1.1751x; 1.1751x over previous
"""Trainium2 Bass kernel for nn_Attention_54150947668207 (sparse channel attention).

Algorithm restructure (verified exact vs reference in fp32, rel 3.4e-7):
  - dwconv3x3 per channel on q,k,v (depthwise, SAME pad)
  - per (batch,head): attn = normalize(q) @ normalize(k)^T over pixels; the 4
    top-k masked softmaxes combine into ONE matrix A_comb = sum_i w_i*softmax_i
    (top-k via rank-count, col-scaled before ranking; exp needs no max-sub)
  - M_combT = blockdiag(A_comb)^T @ Wproj^T folds all four attn@v matmuls AND
    the 1x1 projection into ONE [384,384]@[384,px] matmul per pixel shard.

Sharding (8 cores, SPMD):
  - attention phase: core i handles batch i//4, heads {2*(i%4), 2*(i%4)+1}
  - projection phase: core i handles batch i//4, image rows 32*(i%4)..+32
  - connected by one AllGather of M_combT ([96,384] f16) in 4-core groups.

v2 schedule (from trace analysis of v1 at 280us):
  - PE stream kept dense & in one program order: qk dwconv chunks -> attn
    matmuls -> v-group0 (all-9-taps PE; covers the post-attn DVE/ACT wait) ->
    M matmuls -> v-groups 1,2 -> warm-fill -> final matmuls. Keeps HAM at
    full clock (v1 spent 102us at half clock) and fills the collective window.
  - norm reciprocals + rk broadcast DRAM bounce issued during the attn matmul
    phase (their latency fully hidden).
  - A1 head-1 block extracted straight from PSUM with a base-partition-32
    ACT copy (overwritten rows fixed by the head-0 copy) - no DRAM bounce.
  - diag-weight tables host-pre-transposed -> contiguous DMA loads.
  - input loads prefetched: first chunks on HWDGE (sync), rest on SWDGE
    (gpsimd); all dwconv transposes ride the sync HWDGE ring.
"""
import sys

for _p in ("/opt/trn_rl_repo",):
    if _p not in sys.path:
        sys.path.insert(0, _p)

import numpy as np
from contextlib import ExitStack

import concourse.bass as bass
import concourse.tile as tile
from concourse import mybir
from concourse.bass_utils import run_bass_kernel_spmd

F32 = mybir.dt.float32
F16 = mybir.dt.float16
AOT = mybir.AluOpType
ACTF = mybir.ActivationFunctionType

C = 384
HEADS = 8
CH = 48          # channels per head
H = W = 128
HW = H * W
B = 2
C2 = 96          # channels per core in attention phase (2 heads)
KS = (CH // 2, CH * 2 // 3, CH * 3 // 4, CH * 4 // 5)  # 24, 32, 36, 38

# tap order t = 3*ky + kx, offsets (dy,dx) = (ky-1, kx-1)
N_TAPS = 9
CFG = {
    "pe_taps_qk": (0, 1, 3, 4, 5, 7),   # taps done on PE (fp32 psum acc)
    "pe_taps_v": (0, 1, 3, 4, 5, 7),
    "split_waits": True,
    "max_waits": 1,
    "warm_fill_n": 60,   # junk matmuls bridging the collective window
    "pads_bufs": 3,
}

NCHUNK = 4            # q/k processed in 4 chunks of 32 rows
ROWS_PER_CHUNK = 32
CHUNK_PX = ROWS_PER_CHUNK * W   # 4096
SUB = 512             # psum sub-chunk width for PE dwconv
VROWS = 32            # v shard rows per core
VPX = VROWS * W       # 4096


def _split_multi_waits(nc, max_waits=1):
    """walrus in this container accepts limited sync waits per instruction;
    split extras into preceding single-wait NoOps on the same engine."""
    n = 0
    for f in nc.m.functions:
        for blk in f.blocks:
            new_insts = []
            for inst in blk.instructions:
                si = getattr(inst, "sync_info", None)
                if si is not None and si.on_wait and len(si.on_wait) > max_waits:
                    waits = list(si.on_wait)
                    for wcond in waits[:-max_waits]:
                        nop = mybir.InstNoOp(
                            name=f"I-waitsplit-{nc.next_id()}",
                            ins=[], outs=[],
                            engine=inst.engine,
                            sync_info=mybir.SyncInfo(on_wait=[wcond], on_update=[]),
                        )
                        new_insts.append(nop)
                        n += 1
                    si.on_wait = waits[-max_waits:]
                new_insts.append(inst)
            blk.instructions = new_insts
    return n


def _emit_dwconv(nc, psum_dw, xpad, w9, diags, dwp, out_tag,
                 npart, pe_taps, nrows, out_final=None, tmp_pool=None):
    """Depthwise 3x3 over nrows output rows.

    PE taps accumulate in fp32 PSUM (diag matmuls); remaining taps run as
    DVE tensor_scalar(4x) + tensor_tensor(2x) chains.
    Returns the final output AP ([npart, nrows*W] f16).
    """
    dve_taps = [t for t in range(N_TAPS) if t not in pe_taps]
    npx = nrows * W
    nsub = npx // 1024
    rows_per_sub = 1024 // W  # 8

    def shifted(t, r_lo, nr):
        ky, kx = divmod(t, 3)
        return xpad[:, r_lo + ky:r_lo + ky + nr, kx:kx + W]

    _ctr = [0]

    def alloc(tag, pool=None):
        _ctr[0] += 1
        t = (pool or dwp).tile([npart, npx], F16, tag=tag,
                               name=f"{out_tag}_{tag}{_ctr[0]}")
        return t[:]

    n_dve = len(dve_taps)
    assert pe_taps
    cur = (out_final if (n_dve == 0 and out_final is not None)
           else alloc(out_tag + "A"))
    oc3 = cur.rearrange("p (r w) -> p r w", w=W)
    for s in range(nsub):
        r_lo = s * rows_per_sub
        ps = psum_dw.tile([npart, 1024], F32, tag="psdw")
        for half in range(2):
            for i, t in enumerate(pe_taps):
                nc.tensor.matmul(
                    ps[:, half * 512:half * 512 + 512], diags[t],
                    shifted(t, r_lo + half * 4, 4),
                    start=(i == 0), stop=(i == len(pe_taps) - 1))
        nc.scalar.copy(oc3[:, r_lo:r_lo + rows_per_sub, :], ps[:])
    if n_dve == 0:
        return cur
    flip = 0
    for j, t in enumerate(dve_taps):
        last = (j == n_dve - 1)
        nxt = out_final if (last and out_final is not None) else alloc(out_tag + "BA"[flip])
        flip ^= 1
        no3 = nxt.rearrange("p (r w) -> p r w", w=W)
        tmp = alloc("dwtmp", pool=tmp_pool)
        tm3 = tmp.rearrange("p (r w) -> p r w", w=W)
        nc.vector.tensor_scalar(
            tm3, shifted(t, 0, nrows), w9[:, t:t + 1], None, AOT.mult)
        nc.vector.tensor_tensor(no3, tm3, oc3, AOT.add)
        cur, oc3 = nxt, no3
    return cur


def build_kernel():
    nc = bass.Bass("TRN2", target_bir_lowering=False, debug=False, num_devices=8)

    # ---- DRAM I/O ----
    qs = nc.declare_dram_parameter("qs", [C2, 130, 130], F16, isOutput=False)
    ks = nc.declare_dram_parameter("ks", [C2, 130, 130], F16, isOutput=False)
    vs = nc.declare_dram_parameter("vs", [C, 34, 130], F16, isOutput=False)
    wq9 = nc.declare_dram_parameter("wq9", [C2, 9], F32, isOutput=False)
    wk9 = nc.declare_dram_parameter("wk9", [C2, 9], F32, isOutput=False)
    wv9 = nc.declare_dram_parameter("wv9", [128, 3, 9], F32, isOutput=False)
    # host pre-transposed: [c, a, t, e] and [c, t, ct, e] (all 9 v taps)
    dgqk = nc.declare_dram_parameter("dgqk", [C2, 2, 9, C2], F16, isOutput=False)
    dgv = nc.declare_dram_parameter("dgv", [128, 9, 3, 128], F16, isOutput=False)
    wpT0 = nc.declare_dram_parameter("wpT0", [CH, C], F16, isOutput=False)
    wpT1 = nc.declare_dram_parameter("wpT1", [CH, C], F16, isOutput=False)
    tempv = nc.declare_dram_parameter("tempv", [C2, 1], F32, isOutput=False)
    attwv = nc.declare_dram_parameter("attwv", [C2, 4], F32, isOutput=False)
    out_ext = nc.declare_dram_parameter("out", [3, 128, VPX], F16, isOutput=True)

    with tile.TileContext(nc) as tc, ExitStack() as ctx:
        pool = ctx.enter_context(tc.tile_pool(name="sbuf", bufs=1))
        pads = ctx.enter_context(tc.tile_pool(name="pads", bufs=CFG["pads_bufs"]))
        vpads = ctx.enter_context(tc.tile_pool(name="vpads", bufs=2))
        dwp = ctx.enter_context(tc.tile_pool(name="dwp", bufs=2))
        vwp = ctx.enter_context(tc.tile_pool(name="vwp", bufs=1))
        psum_dw = ctx.enter_context(tc.tile_pool(name="psdw", bufs=2, space="PSUM"))
        psum_a = ctx.enter_context(tc.tile_pool(name="psa", bufs=1, space="PSUM"))
        psum_o = ctx.enter_context(tc.tile_pool(name="pso", bufs=2, space="PSUM"))
        obuf = ctx.enter_context(tc.tile_pool(name="obuf", bufs=3))
        dram = ctx.enter_context(tc.tile_pool(name="dram", bufs=1, space="DRAM"))

        # ---- critical-path loads on HWDGE(sync): diag weights + first chunks
        dgqk_t = pool.tile([C2, 2, 9, C2], F16, tag="dgqk")
        nc.sync.dma_start(dgqk_t[:], dgqk.ap())
        w9q = pool.tile([C2, 9], F32); nc.sync.dma_start(w9q[:], wq9.ap())
        w9k = pool.tile([C2, 9], F32); nc.sync.dma_start(w9k[:], wk9.ap())
        xqk = {}
        for ci in range(2):
            r0 = ci * ROWS_PER_CHUNK
            for nm, src in (("q", qs), ("k", ks)):
                t = pads.tile([C2, 34, 130], F16, tag="pad", name=f"x{nm}{ci}")
                nc.sync.dma_start(t[:], src.ap()[:, r0:r0 + 34, :])
                xqk[(nm, ci)] = t

        # ---- the rest on SWDGE(gpsimd): v inputs, remaining consts, chunks 2-3
        vpad = []
        for ct in range(2):
            vp = vpads.tile([128, 34, 130], F16, tag="vpad")
            nc.gpsimd.dma_start(vp[:], vs.ap()[128 * ct:128 * (ct + 1), :, :])
            vpad.append(vp)
        dgv_t = pool.tile([128, 9, 3, 128], F16, tag="dgvt")
        nc.gpsimd.dma_start(dgv_t[:], dgv.ap())
        w9v = pool.tile([128, 3, 9], F32); nc.gpsimd.dma_start(w9v[:], wv9.ap())
        wp0 = pool.tile([CH, C], F16); nc.gpsimd.dma_start(wp0[:], wpT0.ap())
        wp1 = pool.tile([CH, C], F16); nc.gpsimd.dma_start(wp1[:], wpT1.ap())
        tmpv = pool.tile([C2, 1], F32); nc.gpsimd.dma_start(tmpv[:], tempv.ap())
        attw = pool.tile([C2, 4], F32); nc.gpsimd.dma_start(attw[:], attwv.ap())
        for ci in range(2, NCHUNK):
            r0 = ci * ROWS_PER_CHUNK
            for nm, src in (("q", qs), ("k", ks)):
                t = pads.tile([C2, 34, 130], F16, tag="pad", name=f"x{nm}{ci}")
                nc.gpsimd.dma_start(t[:], src.ap()[:, r0:r0 + 34, :])
                xqk[(nm, ci)] = t
        # v group 2 rides the pads rotation (reuses xq3's slot -> loads once
        # chunk-3 q dwconv is done; needed much later, at v-group-2 time)
        vp2 = pads.tile([128, 34, 130], F16, tag="pad", name="vp2")
        nc.gpsimd.dma_start(vp2[:], vs.ap()[256:384, :, :])
        vpad.append(vp2)

        diag_q = {t: dgqk_t[:, 0, t, :] for t in range(9)}
        diag_k = {t: dgqk_t[:, 1, t, :] for t in range(9)}
        diag_v = {(t, ct): dgv_t[:, t, ct, :]
                  for t in range(9) for ct in range(3)}

        # ---- q/k dwconv, 4 chunks; transposes on sync HWDGE ----
        sumsq = {}
        vdw = pool.tile([128, 3, VPX], F16, tag="vdw")
        qT = pool.tile([128, 128, C2], F16, tag="qT")
        kT = pool.tile([128, 128, C2], F16, tag="kT")
        for ci in range(NCHUNK):
            dws = {}
            for name in ("q", "k"):
                w9_ = w9q if name == "q" else w9k
                dg_ = diag_q if name == "q" else diag_k
                taps = CFG["pe_taps_qk"] if ci < NCHUNK - 1 else tuple(range(9))
                dw = _emit_dwconv(nc, psum_dw, xqk[(name, ci)], w9_, dg_,
                                  dwp, "dw", C2, taps, ROWS_PER_CHUNK)
                dws[name] = dw
                sq = dwp.tile([C2, CHUNK_PX], F16, tag="dwtmp", name=f"sq_{name}{ci}")
                ss = pool.tile([C2, 1], F32, tag=f"ss_{name}{ci}")
                nc.scalar.activation(sq[:], dw, ACTF.Square, accum_out=ss[:])
                sumsq[(name, ci)] = ss
            nc.sync.dma_start_transpose(qT[:, 32 * ci:32 * ci + 32, :], dws["q"])
            nc.sync.dma_start_transpose(kT[:, 32 * ci:32 * ci + 32, :], dws["k"])

        # ---- norms: total sumsq -> rq, rk, rsc, Bc (latency hides under attn)
        nq2 = pool.tile([C2, 1], F32, tag="nq2")
        nk2 = pool.tile([C2, 1], F32, tag="nk2")
        for name, tgt in (("q", nq2), ("k", nk2)):
            nc.vector.tensor_tensor(
                tgt[:], sumsq[(name, 0)][:], sumsq[(name, 1)][:], AOT.add)
            nc.vector.tensor_tensor(
                tgt[:], tgt[:], sumsq[(name, 2)][:], AOT.add)
            nc.vector.tensor_tensor(
                tgt[:], tgt[:], sumsq[(name, 3)][:], AOT.add)
        rq = pool.tile([C2, 1], F32, tag="rq")
        rk = pool.tile([C2, 1], F32, tag="rk")
        for src2, dst in ((nq2, rq), (nk2, rk)):
            nc.scalar.sqrt(dst[:], src2[:])
            nc.vector.reciprocal(dst[:], dst[:])
        rk_dram = dram.tile([C2, 1], F32)
        nc.sync.dma_start(rk_dram[:], rk[:])
        Bc = pool.tile([C2, CH], F32, tag="Bc")
        rkd = rk_dram[:].rearrange("p one -> (p one)")
        nc.sync.dma_start(
            Bc[0:CH, :],
            rkd[0:CH].rearrange("(x e) -> x e", x=1).broadcast_to([CH, CH]))
        nc.sync.dma_start(
            Bc[CH:C2, :],
            rkd[CH:C2].rearrange("(x e) -> x e", x=1).broadcast_to([CH, CH]))
        rsc = pool.tile([C2, 1], F32, tag="rsc")
        nc.vector.tensor_tensor(rsc[:], rq[:], tmpv[:], AOT.mult)

        # ---- attention matmuls: one dense PE stream over all 128 px groups
        ps_attn = psum_a.tile([C2, C2], F32, tag="psattn")
        for j in range(128):
            nc.tensor.matmul(ps_attn[:], qT[:, j, :], kT[:, j, :],
                             start=(j == 0), stop=(j == 127))

        # ---- post-attention (small, DVE/ACT) ----
        # A1[r, d] = attn[r, head(r)*48 + d]; head1 block via base-partition-32
        # psum copy (rows 32:48 garbage, then overwritten by the head0 copy)
        A1 = pool.tile([C2, CH], F32, tag="A1")
        nc.scalar.copy(A1[32:64, :], ps_attn[32:64, CH:C2])
        nc.scalar.copy(A1[64:C2, :], ps_attn[64:C2, CH:C2])
        nc.scalar.copy(A1[0:CH, :], ps_attn[0:CH, 0:CH])
        nc.vector.tensor_tensor(A1[:], A1[:], Bc[:], AOT.mult)
        # E before G: ACT exponentiates while DVE does the rank count
        E = pool.tile([C2, CH], F32, tag="E")
        nc.scalar.activation(E[:], A1[:], ACTF.Exp, scale=rsc[:])
        # rank count: G[r, d, e] = A1[r, e] > A1[r, d]  (free dims d,e)
        G = pool.tile([C2, CH, CH], F16, tag="G")
        in_e = A1[:].rearrange("p (x e) -> p x e", x=1).broadcast_to([C2, CH, CH])
        in_d = A1[:].rearrange("p (d x) -> p d x", x=1).broadcast_to([C2, CH, CH])
        nc.vector.tensor_tensor(G[:], in_e, in_d, AOT.is_gt)
        cnt = pool.tile([C2, CH], F32, tag="cnt")
        nc.vector.tensor_reduce(cnt[:], G[:], axis=mybir.AxisListType.X, op=AOT.add)
        # 4 masked softmaxes combined; Acc kept f16 so head blocks feed matmul
        Acc = pool.tile([C2, CH], F16, tag="Acc")
        numer = pool.tile([C2, CH], F32, tag="numer")
        for i, kk in enumerate(KS):
            mi = pool.tile([C2, CH], F16, tag="mi")
            nc.vector.tensor_scalar(mi[:], cnt[:], float(kk), None, AOT.is_lt)
            nc.vector.tensor_tensor(numer[:], E[:], mi[:], AOT.mult)
            den = pool.tile([C2, 1], F32, tag="den")
            nc.vector.tensor_reduce(den[:], numer[:], axis=mybir.AxisListType.X,
                                    op=AOT.add)
            rw = pool.tile([C2, 1], F32, tag="rw")
            nc.vector.reciprocal(rw[:], den[:])
            nc.vector.tensor_tensor(rw[:], rw[:], attw[:, i:i + 1], AOT.mult)
            if i == 0:
                nc.vector.tensor_scalar(Acc[:], numer[:], rw[:], None, AOT.mult)
            else:
                nc.vector.scalar_tensor_tensor(
                    Acc[:], numer[:], rw[:], Acc[:], AOT.mult, AOT.add)
        # head1 rows to base partition 0 (head0 is Acc[0:CH] in place)
        Ah1 = pool.tile([CH, CH], F16, tag="Ah1")
        nc.gpsimd.dma_start(Ah1[:], Acc[CH:C2, :])

        # ---- v dwconv group 0: all-PE taps, fills the post-attn PE window
        _emit_dwconv(nc, psum_dw, vpad[0], w9v[:, 0, :],
                     {t: diag_v[(t, 0)] for t in range(9)}, vwp, "vw",
                     128, tuple(range(9)), VROWS, out_final=vdw[:, 0, :])

        # ---- M_combT partials -> DRAM -> AllGather within 4-core groups
        b_in = dram.tile([C2, C], F16)
        b_out = dram.tile([4, C2, C], F16)
        for h, (ah, wp) in enumerate(((Acc[0:CH, :], wp0), (Ah1[:], wp1))):
            ps = psum_a.tile([CH, C], F32, tag="psmc")
            nc.tensor.matmul(ps[:], ah, wp[:], start=True, stop=True)
            mt_h = pool.tile([CH, C], F16, tag=f"mth{h}")
            nc.scalar.copy(mt_h[:], ps[:])
            nc.sync.dma_start(b_in[CH * h:CH * (h + 1), :], mt_h[:])
        nc.gpsimd.collective_compute(
            "AllGather", AOT.bypass,
            replica_groups=[[0, 1, 2, 3], [4, 5, 6, 7]],
            ins=[b_in.opt()], outs=[b_out.opt()])

        # ---- v dwconv groups 1,2 (PE + DVE taps) during the collective
        for ct in (1, 2):
            diags_v = {t: diag_v[(t, ct)] for t in CFG["pe_taps_v"]}
            _emit_dwconv(nc, psum_dw, vpad[ct], w9v[:, ct, :],
                         diags_v, vwp, "vw", 128, CFG["pe_taps_v"], VROWS,
                         out_final=vdw[:, ct, :], tmp_pool=dwp)

        # ---- warm-fill: junk matmuls keep HAM up while AllGather completes
        for i in range(CFG["warm_fill_n"]):
            warm_ps = psum_o.tile([128, 512], F32, tag="psout", name=f"warm{i}")
            nc.tensor.matmul(
                warm_ps[:], dgv_t[:, 0, 0, :],
                vdw[:, i % 3, 512 * (i % 8):512 * (i % 8) + 512],
                start=True, stop=True)

        # ---- gather result + final matmul ----
        MT = pool.tile([128, 3, C], F16, tag="MT")
        bo = b_out[:].rearrange("g p c -> (g p) c")
        for kc in range(3):
            nc.sync.dma_start(MT[:, kc, :], bo[128 * kc:128 * (kc + 1), :])
        for m in range(3):
            for n in range(VPX // SUB):
                ps = psum_o.tile([128, SUB], F32, tag="psout")
                for kc in range(3):
                    nc.tensor.matmul(
                        ps[:], MT[:, kc, 128 * m:128 * (m + 1)],
                        vdw[:, kc, SUB * n:SUB * (n + 1)],
                        start=(kc == 0), stop=(kc == 2))
                ob = obuf.tile([128, SUB], F16, tag="ob")
                nc.scalar.copy(ob[:], ps[:])
                nc.sync.dma_start(out_ext.ap()[m, :, SUB * n:SUB * (n + 1)], ob[:])

    if CFG["split_waits"]:
        _split_multi_waits(nc, CFG["max_waits"])
    return nc


# ---------------- host-side sharding ----------------

def _prep_inputs(q_fea, k_fea, v_fea, wq, wk, wv, wproj, temperature, attn_w):
    q_fea = np.asarray(q_fea, np.float32)
    k_fea = np.asarray(k_fea, np.float32)
    v_fea = np.asarray(v_fea, np.float32)
    wq = np.asarray(wq, np.float32)[:, 0]      # [C,3,3]
    wk = np.asarray(wk, np.float32)[:, 0]
    wv = np.asarray(wv, np.float32)[:, 0]
    wproj = np.asarray(wproj, np.float32)[:, :, 0, 0]  # [C,C]
    temp = np.asarray(temperature, np.float32).reshape(HEADS)
    attn_w = np.asarray(attn_w, np.float32).reshape(4)

    wq9 = wq.reshape(C, 9)
    wk9 = wk.reshape(C, 9)
    wv9 = wv.reshape(C, 9)

    # dgv host layout [c, t, ct, e] (all 9 taps)
    dgv = np.zeros((128, 9, 3, 128), np.float16)
    for t in range(9):
        for ct in range(3):
            w = wv9[128 * ct:128 * (ct + 1), t].astype(np.float16)
            dgv[np.arange(128), t, ct, np.arange(128)] = w

    in_maps = []
    for core in range(8):
        b = core // 4
        g = core % 4
        ch0 = C2 * g
        r0 = VROWS * g

        def padqk(x):
            p = np.zeros((C2, 130, 130), np.float16)
            p[:, 1:129, 1:129] = x[b, ch0:ch0 + C2]
            return p

        vp = np.zeros((C, 34, 130), np.float16)
        glo = max(0, r0 - 1)
        ghi = min(H, r0 + VROWS + 1)
        vp[:, glo - (r0 - 1):ghi - (r0 - 1), 1:129] = v_fea[b, :, glo:ghi, :]

        # dgqk host layout [c, a, t, e]
        dgqk = np.zeros((C2, 2, 9, C2), np.float16)
        for t in range(9):
            dgqk[np.arange(C2), 0, t, np.arange(C2)] = \
                wq9[ch0:ch0 + C2, t].astype(np.float16)
            dgqk[np.arange(C2), 1, t, np.arange(C2)] = \
                wk9[ch0:ch0 + C2, t].astype(np.float16)

        in_maps.append({
            "qs": padqk(q_fea),
            "ks": padqk(k_fea),
            "vs": vp,
            "wq9": np.ascontiguousarray(wq9[ch0:ch0 + C2]),
            "wk9": np.ascontiguousarray(wk9[ch0:ch0 + C2]),
            "wv9": np.ascontiguousarray(wv9.reshape(3, 128, 9).transpose(1, 0, 2)),
            "dgqk": dgqk,
            "dgv": dgv,
            "wpT0": np.ascontiguousarray(wproj[:, ch0:ch0 + CH].T.astype(np.float16)),
            "wpT1": np.ascontiguousarray(wproj[:, ch0 + CH:ch0 + C2].T.astype(np.float16)),
            "tempv": np.repeat(temp[2 * g:2 * g + 2], CH)[:, None].copy(),
            "attwv": np.tile(attn_w, (C2, 1)),
        })
    return in_maps


def _assemble(results):
    out = np.zeros((B, C, H, W), np.float32)
    for core in range(8):
        b = core // 4
        r0 = VROWS * (core % 4)
        o = results[core]["out"].astype(np.float32)  # [3, 128, VPX]
        out[b, :, r0:r0 + VROWS, :] = o.reshape(C, VROWS, W)
    return out


_CACHE = {}


def kernel(**inputs) -> np.ndarray:
    if "nc" not in _CACHE:
        _CACHE["nc"] = build_kernel()
    nc = _CACHE["nc"]
    in_maps = _prep_inputs(**inputs)
    res = run_bass_kernel_spmd(nc, in_maps, core_ids=list(range(8)))
    return _assemble(res.results)


if __name__ == "__main__":
    sys.path.insert(0, "/root/problem")
    from reference import setup_inputs, reference

    inputs = setup_inputs()
    ref = np.asarray(reference(**inputs))
    got = kernel(**{k: np.asarray(v) for k, v in inputs.items()})
    rel = np.linalg.norm(got - ref) / np.linalg.norm(ref)
    print(f"Relative error: {rel:.3e}")
